# revision 1
# baseline (speedup 1.0000x reference)
"""Multi-head self-attention Trainium2 kernel (8 NeuronCores).

Sharding: 8 cores = 4 batches x 2 head-groups (8 heads each).
Core c handles batch b=c//2, heads [g*8, (g+1)*8) where g=c%2.
Each core computes a partial output (its heads' contribution to the
output projection); the host sums the two partials per batch and adds bo.

All matmuls run in float32r (fp32 data, ~1 cycle/row vs 4 for fp32,
~1.5e-4 matmul rel err). fp32r matmuls require output base partition 0.

Per-core dataflow:
  xT [1024, 2048] (= x[b].T), wq/wk/wv [1024, 512], wo [512, 1024]
  A1: QT[p]/KT[p] = w_p.T @ x.T  [128, 2048] per head-pair p (2 heads x 64
      dims on partitions). PSUM accum over 8 k-tiles.
  A2: VS[jt] = [x_jt @ wv | ones] per 128-token tile: [128, 8*65] with a
      ones column per head (the ones column makes the PV matmul emit the
      softmax normalizer as row 64 of the context tile).
  B:  per (pair p, 512-query block qb):
        ST[j-tile, i] = KT_h-slice.T x QT_h  (K=64, head pair row-packed)
        PT = exp(0.125 * ST)                 (ScalarE, 1536/1024-elem groups)
        ct_par[c(65), i] += VS[jt]_h.T @ PT  (row 64 accumulates sum(exp))
        normalize: recip(row64) -> K=1 matmul broadcast -> DVE mult
        -> cth[h] [64, 512] per head (fp32r)
  C:  per qb: out[tokens, :] = sum_h cth[h].T-slice @ wo_h  (K=64 accum)
"""

import numpy as np

import concourse.bass as bass
import concourse.tile as tile
from concourse import bacc, mybir
from contextlib import ExitStack

P = 128
D = 1024
HD = 512  # head dims per core (8 heads x 64)
NPAIR = 4
NH = 8
F32 = mybir.dt.float32
FR = mybir.dt.float32r


def _st_groups(n_slices):
    """Split n_slices exp slices into alternating groups of 3 and 2."""
    groups = []
    want = 3
    rem = n_slices
    while rem > 0:
        g = min(want, rem)
        groups.append(g)
        rem -= g
        want = 2 if want == 3 else 3
    return groups


def build_nc(S=2048):
    NKT = D // P          # 8 k-tiles over model dim
    NJT = S // P          # key tiles
    MSEG = 512
    NMSEG = S // MSEG
    QB = 512
    NQB = S // QB

    nc = bacc.Bacc("TRN2", target_bir_lowering=False, debug=False)
    xT = nc.dram_tensor("xT", [D, S], FR, kind="ExternalInput").ap()
    wq = nc.dram_tensor("wq", [D, HD], FR, kind="ExternalInput").ap()
    wk = nc.dram_tensor("wk", [D, HD], FR, kind="ExternalInput").ap()
    wv = nc.dram_tensor("wv", [D, HD], FR, kind="ExternalInput").ap()
    wo = nc.dram_tensor("wo", [HD, D], FR, kind="ExternalInput").ap()
    out = nc.dram_tensor("out", [S, D], F32, kind="ExternalOutput").ap()

    with tile.TileContext(nc) as tc:
        with ExitStack() as persist:
            const_pool = persist.enter_context(tc.tile_pool(name="const", bufs=1))
            data_pool = persist.enter_context(tc.tile_pool(name="data", bufs=1))

            ones_f32 = const_pool.tile([P, 64], F32, tag="ones32", name="ones_f32")
            nc.vector.memset(ones_f32[:], 1.0)
            ones = const_pool.tile([P, 64], FR, tag="ones", name="ones")
            nc.vector.tensor_copy(ones[:], ones_f32[:])
            ones8_f32 = const_pool.tile([P, NH], F32, tag="ones8", name="ones8_f32")
            nc.vector.memset(ones8_f32[:], 1.0)

            QT = [data_pool.tile([P, S], FR, tag=f"qt{p}", name=f"qt{p}")
                  for p in range(NPAIR)]
            KT = [data_pool.tile([P, S], FR, tag=f"kt{p}", name=f"kt{p}")
                  for p in range(NPAIR)]
            # [128 tokens, 8 heads x (64 dims + ones col)]
            VS = [data_pool.tile([P, NH * 65], FR, tag=f"vs{j}", name=f"vs{j}")
                  for j in range(NJT)]

            # ---------------- Phase A: projections ----------------
            with ExitStack() as es_a:
                w_pool = es_a.enter_context(tc.tile_pool(name="wpool", bufs=1))
                chunk_pool = es_a.enter_context(tc.tile_pool(name="chunks", bufs=6))

                wq_t = w_pool.tile([P, NKT, HD], FR, tag="wq", name="wq_t")
                nc.sync.dma_start(wq_t[:], wq.rearrange("(kt p) n -> p kt n", p=P))
                wk_t = w_pool.tile([P, NKT, HD], FR, tag="wk", name="wk_t")
                nc.sync.dma_start(wk_t[:], wk.rearrange("(kt p) n -> p kt n", p=P))
                wv_t = w_pool.tile([P, NKT, HD], FR, tag="wv", name="wv_t")
                nc.sync.dma_start(wv_t[:], wv.rearrange("(kt p) n -> p kt n", p=P))

                # --- A1: QT / KT (8 PSUM accumulators: (q|k) x 4 pairs) ---
                with tc.tile_pool(name="qkps", bufs=8, space="PSUM") as qk_pool:
                    for mseg in range(NMSEG):
                        accs = [qk_pool.tile([P, MSEG], F32, tag="qk", name="qkacc")
                                for _ in range(8)]
                        for kt in range(NKT):
                            xc = chunk_pool.tile([P, MSEG], FR, tag="xc", name="xc")
                            nc.sync.dma_start(
                                xc[:],
                                xT[kt * P:(kt + 1) * P, mseg * MSEG:(mseg + 1) * MSEG])
                            for p in range(NPAIR):
                                for ti, wt in ((0, wq_t), (1, wk_t)):
                                    nc.tensor.matmul(
                                        accs[p * 2 + ti][:],
                                        lhsT=wt[:, kt, p * P:(p + 1) * P],
                                        rhs=xc[:],
                                        start=(kt == 0), stop=(kt == NKT - 1))
                        for p in range(NPAIR):
                            nc.vector.tensor_copy(
                                QT[p][:, mseg * MSEG:(mseg + 1) * MSEG], accs[p * 2][:])
                            nc.vector.tensor_copy(
                                KT[p][:, mseg * MSEG:(mseg + 1) * MSEG], accs[p * 2 + 1][:])

                # --- A2: V (natural layout, 4 j-tiles per mseg) ---
                with tc.tile_pool(name="vps", bufs=8, space="PSUM") as v_pool:
                    for mseg in range(NMSEG):
                        vaccs = [v_pool.tile([P, HD], F32, tag="v", name="vacc")
                                 for _ in range(4)]
                        for kt in range(NKT):
                            xc = chunk_pool.tile([P, MSEG], FR, tag="xc", name="xc")
                            nc.sync.dma_start(
                                xc[:],
                                xT[kt * P:(kt + 1) * P, mseg * MSEG:(mseg + 1) * MSEG])
                            for i in range(4):
                                nc.tensor.matmul(
                                    vaccs[i][:],
                                    lhsT=xc[:, i * P:(i + 1) * P],
                                    rhs=wv_t[:, kt, :],
                                    start=(kt == 0), stop=(kt == NKT - 1))
                        for i in range(4):
                            vsv = VS[mseg * 4 + i].rearrange("p (h c) -> p h c", c=65)
                            nc.vector.tensor_copy(vsv[:, :, 0:64], vaccs[i][:])
                            nc.vector.tensor_copy(vsv[:, :, 64], ones8_f32[:])

            # ---------------- Phases B + C: attention + projection ----------------
            with ExitStack() as es_b:
                cth_pool = es_b.enter_context(tc.tile_pool(name="cthpool", bufs=2))
                wo_pool = es_b.enter_context(tc.tile_pool(name="wopool", bufs=1))
                pt_pool = es_b.enter_context(tc.tile_pool(name="ptpool", bufs=2))
                rc_pool = es_b.enter_context(tc.tile_pool(name="rcpool", bufs=2))
                po_pool = es_b.enter_context(tc.tile_pool(name="popool", bufs=3))
                st_ps = es_b.enter_context(tc.tile_pool(name="stps", bufs=1, space="PSUM"))
                ct_ps = es_b.enter_context(tc.tile_pool(name="ctps", bufs=1, space="PSUM"))
                pj_ps = es_b.enter_context(tc.tile_pool(name="pjps", bufs=1, space="PSUM"))

                wo_h = []
                for h in range(NH):
                    t = wo_pool.tile([64, D], FR, tag=f"wo{h}", name=f"wo{h}")
                    nc.sync.dma_start(t[:], wo[h * 64:(h + 1) * 64, :])
                    wo_h.append(t)

                groups = _st_groups(2 * NJT)

                for qb in range(NQB):
                    cth = [None] * NH
                    for p in range(NPAIR):
                        cts = [ct_ps.tile([65, QB], F32, tag="cte", name="cte"),
                               ct_ps.tile([65, QB], F32, tag="cto", name="cto")]
                        s0 = 0
                        for gl in groups:
                            tag = "stA" if gl == 3 else "stB"
                            stg = st_ps.tile([P, gl * 512], F32, tag=tag, name="stg")
                            for l in range(gl):
                                s = s0 + l
                                jt, par = divmod(s, 2)
                                nc.tensor.matmul(
                                    stg[:, l * 512:(l + 1) * 512],
                                    lhsT=KT[p][par * 64:(par + 1) * 64,
                                               jt * P:(jt + 1) * P],
                                    rhs=QT[p][par * 64:(par + 1) * 64,
                                              qb * QB:(qb + 1) * QB],
                                    start=True, stop=True)
                            ptg = pt_pool.tile([P, gl * 512], FR, tag=tag, name="ptg")
                            nc.scalar.activation(
                                ptg[:], stg[:],
                                mybir.ActivationFunctionType.Exp, scale=0.125)
                            for l in range(gl):
                                s = s0 + l
                                jt, par = divmod(s, 2)
                                h = 2 * p + par
                                nc.tensor.matmul(
                                    cts[par][:],
                                    lhsT=VS[jt][:, h * 65:(h + 1) * 65],
                                    rhs=ptg[:, l * 512:(l + 1) * 512],
                                    start=(jt == 0), stop=(jt == NJT - 1))
                            s0 += gl

                        # normalize: cth[h] = ct[0:64] * (1/ct[64]) broadcast
                        for par in range(2):
                            h = 2 * p + par
                            ct = cts[par]
                            rc = rc_pool.tile([65, QB], FR, tag="rc", name="rc")
                            with nc.allow_low_precision(reason="softmax recip"):
                                nc.vector.reciprocal(rc[64:65, :], ct[64:65, :])
                            bc_ps = pj_ps.tile([P, QB], F32, tag="pj", name="bc_ps")
                            nc.tensor.matmul(bc_ps[0:64, :], lhsT=ones[64:65, :],
                                             rhs=rc[64:65, :], start=True, stop=True)
                            bc_sb = rc_pool.tile([64, QB], F32, tag="bc", name="bc_sb")
                            nc.vector.tensor_copy(bc_sb[:], bc_ps[0:64, :])
                            t = cth_pool.tile([64, QB], FR, tag=f"cth{h}",
                                              name=f"cth{h}")
                            nc.vector.tensor_tensor(
                                t[:], ct[0:64, :], bc_sb[:], mybir.AluOpType.mult)
                            cth[h] = t

                    # --- Phase C: project this query block's tokens ---
                    for mtl in range(4):
                        mt = qb * 4 + mtl
                        for half in range(2):
                            po = pj_ps.tile([P, 512], F32, tag="pj", name="po")
                            for h in range(NH):
                                nc.tensor.matmul(
                                    po[:],
                                    lhsT=cth[h][:, mtl * P:(mtl + 1) * P],
                                    rhs=wo_h[h][:, half * 512:(half + 1) * 512],
                                    start=(h == 0), stop=(h == NH - 1))
                            po_sb = po_pool.tile([P, 512], F32, tag="posb", name="po_sb")
                            nc.vector.tensor_copy(po_sb[:], po[:])
                            nc.sync.dma_start(
                                out[mt * P:(mt + 1) * P, half * 512:(half + 1) * 512],
                                po_sb[:])
    nc.compile()
    return nc


_NC_CACHE = {}


def _get_nc(S=2048):
    if S not in _NC_CACHE:
        _NC_CACHE[S] = build_nc(S)
    return _NC_CACHE[S]


def kernel(x, Wq, Wk, Wv, Wo, bo):
    from concourse.bass_utils import run_bass_kernel_spmd

    x = np.asarray(x, dtype=np.float32)
    Wq = np.asarray(Wq, dtype=np.float32)
    Wk = np.asarray(Wk, dtype=np.float32)
    Wv = np.asarray(Wv, dtype=np.float32)
    Wo = np.asarray(Wo, dtype=np.float32)
    bo = np.asarray(bo, dtype=np.float32)

    bs, S, d = x.shape
    nc = _get_nc(S)

    in_maps = []
    for c in range(8):
        b, g = divmod(c, 2)
        cols = slice(g * HD, (g + 1) * HD)
        in_maps.append({
            "xT": np.ascontiguousarray(x[b].T),
            "wq": np.ascontiguousarray(Wq[:, cols]),
            "wk": np.ascontiguousarray(Wk[:, cols]),
            "wv": np.ascontiguousarray(Wv[:, cols]),
            "wo": np.ascontiguousarray(Wo[cols, :]),
        })

    res = run_bass_kernel_spmd(nc, in_maps, core_ids=list(range(8)))
    outp = np.empty((bs, S, d), dtype=np.float32)
    for b in range(bs):
        outp[b] = res.results[2 * b]["out"] + res.results[2 * b + 1]["out"] + bo
    return outp



# revision 5
# speedup vs baseline: 1.2332x; 1.2332x over previous
"""Multi-head self-attention Trainium2 kernel (8 NeuronCores).

Sharding: 8 cores = 4 batches x 2 head-groups (8 heads each).
Core c handles batch b=c//2, heads [g*8, (g+1)*8) where g=c%2.
Each core computes a partial output (its heads' contribution to the
output projection); the host sums the two partials per batch and adds bo.

All matmuls run in float32r (fp32 data, ~1 cycle/row). fp32r matmuls
require output base partition 0.

Per-core dataflow:
  xT [1024, 2048] (= x[b].T), wq/wk/wv [1024, 512], wo [512, 1024]
  A1: QT[p]/KT[p] = w_p.T @ x.T  [128, 2048] per head-pair p (2 heads x 64
      dims on partitions). PSUM accum over 8 k-tiles.
  A2: VS[jt] = [x_jt @ wv | ones] per 128-token tile: [128, 8*65] with a
      ones column per head (the ones column makes the PV matmul emit the
      softmax normalizer as row 64 of the context tile).
  B:  per (pair p, 512-query block qb), per key tile jt:
        ST pair: two concurrent row-group matmuls (K=64 at array rows 0-63
        and 64-127) -> st [128, 2, 512] PSUM (2 banks)
        PT = exp(0.125 * ST)  (one ScalarE act over 1024 cols)
        PV: ct_par[65, 512] += VS[jt]_h.T @ PT slot  (row 64 = sum of exp)
      pair tail: copy ct -> SBUF (ctu, 65 rows); DMA row 64 into a packed
      Z tile [8, 512] (one row per head).
  Normalize+project for block qb are DEFERRED and emitted interleaved
  into block qb+1's attention stream (keeps the PE dense and HAM warm):
      one batched reciprocal [8, 512] per qb, one-hot K=8 matmul
      broadcast -> bc [64,512], DVE multiply (in place on ctu),
      then out[tokens, :] accumulated over 8 heads (K=64) per mt/half.
"""

import numpy as np

import concourse.bass as bass
import concourse.tile as tile
from concourse import bacc, mybir
from contextlib import ExitStack

P = 128
D = 1024
HD = 512  # head dims per core (8 heads x 64)
NPAIR = 4
NH = 8
F32 = mybir.dt.float32
FR = mybir.dt.float32r


def build_nc(S=2048):
    NKT = D // P          # 8 k-tiles over model dim
    NJT = S // P          # 16 key tiles
    MSEG = 512
    NMSEG = S // MSEG
    QB = 512
    NQB = S // QB

    nc = bacc.Bacc("TRN2", target_bir_lowering=False, debug=False)
    xT = nc.dram_tensor("xT", [D, S], FR, kind="ExternalInput").ap()
    wq = nc.dram_tensor("wq", [D, HD], FR, kind="ExternalInput").ap()
    wk = nc.dram_tensor("wk", [D, HD], FR, kind="ExternalInput").ap()
    wv = nc.dram_tensor("wv", [D, HD], FR, kind="ExternalInput").ap()
    wo = nc.dram_tensor("wo", [HD, D], FR, kind="ExternalInput").ap()
    sel = nc.dram_tensor("sel", [NH, NH * 64], FR, kind="ExternalInput").ap()
    out = nc.dram_tensor("out", [S, D], F32, kind="ExternalOutput").ap()

    with tile.TileContext(nc) as tc:
        with ExitStack() as persist:
            const_pool = persist.enter_context(tc.tile_pool(name="const", bufs=1))
            data_pool = persist.enter_context(tc.tile_pool(name="data", bufs=1))

            # one-hot selector rows for the Z-broadcast matmul:
            # onehot[j, h*64+m] = (j == h), so lhsT=onehot[:, h*64:(h+1)*64]
            # with rhs=rc[8, 512] broadcasts rc row h onto 64 partitions.
            # (loaded from host: engines can't write at partition offsets 1-7)
            onehot = const_pool.tile([NH, NH * 64], FR, tag="oh", name="onehot")
            nc.sync.dma_start(onehot[:], sel[:])
            ones8_f32 = const_pool.tile([P, NH], F32, tag="ones8", name="ones8_f32")
            nc.vector.memset(ones8_f32[:], 1.0)

            QT = [data_pool.tile([P, S], FR, tag=f"qt{p}", name=f"qt{p}")
                  for p in range(NPAIR)]
            KT = [data_pool.tile([P, S], FR, tag=f"kt{p}", name=f"kt{p}")
                  for p in range(NPAIR)]
            # [128 tokens, 8 heads x (64 dims + ones col)]
            VS = [data_pool.tile([P, NH * 65], FR, tag=f"vs{j}", name=f"vs{j}")
                  for j in range(NJT)]

            # ---------------- Phase A: projections ----------------
            with ExitStack() as es_a:
                w_pool = es_a.enter_context(tc.tile_pool(name="wpool", bufs=1))
                chunk_pool = es_a.enter_context(tc.tile_pool(name="chunks", bufs=6))

                # per-kt weight tiles so the first matmuls start after ~3
                # small DMAs instead of 3 x 2MB ones
                wq_t, wk_t, wv_t = [], [], []
                for kt in range(NKT):
                    tq = w_pool.tile([P, HD], FR, tag=f"wq{kt}", name=f"wq{kt}")
                    nc.sync.dma_start(tq[:], wq[kt * P:(kt + 1) * P, :])
                    wq_t.append(tq)
                    tk = w_pool.tile([P, HD], FR, tag=f"wk{kt}", name=f"wk{kt}")
                    nc.sync.dma_start(tk[:], wk[kt * P:(kt + 1) * P, :])
                    wk_t.append(tk)
                for kt in range(NKT):
                    tv = w_pool.tile([P, HD], FR, tag=f"wv{kt}", name=f"wv{kt}")
                    nc.sync.dma_start(tv[:], wv[kt * P:(kt + 1) * P, :])
                    wv_t.append(tv)

                # --- A1: QT / KT (8 PSUM accumulators: (q|k) x 4 pairs) ---
                with tc.tile_pool(name="qkps", bufs=8, space="PSUM") as qk_pool:
                    for mseg in range(NMSEG):
                        accs = [qk_pool.tile([P, MSEG], F32, tag="qk", name="qkacc")
                                for _ in range(8)]
                        for kt in range(NKT):
                            xc = chunk_pool.tile([P, MSEG], FR, tag="xc", name="xc")
                            nc.sync.dma_start(
                                xc[:],
                                xT[kt * P:(kt + 1) * P, mseg * MSEG:(mseg + 1) * MSEG])
                            for p in range(NPAIR):
                                for ti, wt in ((0, wq_t), (1, wk_t)):
                                    nc.tensor.matmul(
                                        accs[p * 2 + ti][:],
                                        lhsT=wt[kt][:, p * P:(p + 1) * P],
                                        rhs=xc[:],
                                        start=(kt == 0), stop=(kt == NKT - 1))
                        for p in range(NPAIR):
                            nc.vector.tensor_copy(
                                QT[p][:, mseg * MSEG:(mseg + 1) * MSEG], accs[p * 2][:])
                            nc.vector.tensor_copy(
                                KT[p][:, mseg * MSEG:(mseg + 1) * MSEG], accs[p * 2 + 1][:])

                # --- A2: V (natural layout, 4 j-tiles per mseg) ---
                with tc.tile_pool(name="vps", bufs=8, space="PSUM") as v_pool:
                    for mseg in range(NMSEG):
                        vaccs = [v_pool.tile([P, HD], F32, tag="v", name="vacc")
                                 for _ in range(4)]
                        for kt in range(NKT):
                            xc = chunk_pool.tile([P, MSEG], FR, tag="xc", name="xc")
                            nc.sync.dma_start(
                                xc[:],
                                xT[kt * P:(kt + 1) * P, mseg * MSEG:(mseg + 1) * MSEG])
                            for i in range(4):
                                nc.tensor.matmul(
                                    vaccs[i][:],
                                    lhsT=xc[:, i * P:(i + 1) * P],
                                    rhs=wv_t[kt][:],
                                    start=(kt == 0), stop=(kt == NKT - 1))
                        for i in range(4):
                            vsv = VS[mseg * 4 + i].rearrange("p (h c) -> p h c", c=65)
                            nc.vector.tensor_copy(vsv[:, :, 0:64], vaccs[i][:])
                            nc.vector.tensor_copy(vsv[:, :, 64], ones8_f32[:])

            # ---------------- Phases B + C: attention + projection ----------------
            with ExitStack() as es_b:
                wo_pool = es_b.enter_context(tc.tile_pool(name="wopool", bufs=1))
                pt_pool = es_b.enter_context(tc.tile_pool(name="ptpool", bufs=3))
                ctu_pool = es_b.enter_context(tc.tile_pool(name="ctupool", bufs=2))
                z_pool = es_b.enter_context(tc.tile_pool(name="zpool", bufs=2))
                po_pool = es_b.enter_context(tc.tile_pool(name="popool", bufs=3))
                st_ps = es_b.enter_context(tc.tile_pool(name="stps", bufs=2, space="PSUM"))
                ct_ps = es_b.enter_context(tc.tile_pool(name="ctps", bufs=1, space="PSUM"))
                bc_ps = es_b.enter_context(tc.tile_pool(name="bcps", bufs=1, space="PSUM"))
                po_ps = es_b.enter_context(tc.tile_pool(name="pops", bufs=1, space="PSUM"))

                wo_h = []
                for h in range(NH):
                    t = wo_pool.tile([64, D], FR, tag=f"wo{h}", name=f"wo{h}")
                    nc.sync.dma_start(t[:], wo[h * 64:(h + 1) * 64, :])
                    wo_h.append(t)

                pending = []  # deferred normalize+project chunks (closures)

                def emit_pending():
                    if pending:
                        pending.pop(0)()

                for qb in range(NQB):
                    zq = z_pool.tile([NH, QB], FR, tag="z", name="zq")
                    ctu_qb = [None] * NH
                    for p in range(NPAIR):
                        h0, h1 = 2 * p, 2 * p + 1
                        cte = ct_ps.tile([65, QB], F32, tag="cte", name="cte")
                        cto = ct_ps.tile([65, QB], F32, tag="cto", name="cto")
                        qs = slice(qb * QB, (qb + 1) * QB)
                        for jt in range(NJT):
                            js = slice(jt * P, (jt + 1) * P)
                            stg = st_ps.tile([P, 2, MSEG], F32, tag="st", name="stg")
                            # two concurrent row-group matmuls (K=64)
                            nc.tensor.matmul(
                                stg[:, 0, :],
                                lhsT=KT[p][0:64, js], rhs=QT[p][0:64, qs],
                                start=True, stop=True)
                            nc.tensor.matmul(
                                stg[:, 1, :],
                                lhsT=KT[p][64:128, js], rhs=QT[p][64:128, qs],
                                start=True, stop=True)
                            ptg = pt_pool.tile([P, 2, MSEG], FR, tag="pt", name="ptg")
                            nc.scalar.activation(
                                ptg[:], stg[:],
                                mybir.ActivationFunctionType.Exp, scale=0.125)
                            nc.tensor.matmul(
                                cte[:],
                                lhsT=VS[jt][:, h0 * 65:(h0 + 1) * 65],
                                rhs=ptg[:, 0, :],
                                start=(jt == 0), stop=(jt == NJT - 1))
                            nc.tensor.matmul(
                                cto[:],
                                lhsT=VS[jt][:, h1 * 65:(h1 + 1) * 65],
                                rhs=ptg[:, 1, :],
                                start=(jt == 0), stop=(jt == NJT - 1))
                            if jt % 2 == 1 and p < 2:
                                emit_pending()
                        # pair tail: pull context (and Z rows) out of PSUM
                        ctu_e = ctu_pool.tile([65, QB], FR, tag=f"ctu{h0}",
                                              name=f"ctu{h0}")
                        nc.vector.tensor_copy(ctu_e[:], cte[:])
                        ctu_o = ctu_pool.tile([65, QB], FR, tag=f"ctu{h1}",
                                              name=f"ctu{h1}")
                        nc.vector.tensor_copy(ctu_o[:], cto[:])
                        ctu_qb[h0], ctu_qb[h1] = ctu_e, ctu_o
                        # pack Z rows (partition 64 -> partition h) for one
                        # batched reciprocal per qb
                        nc.sync.dma_start(zq[h0:h0 + 1, :], ctu_e[64:65, :])
                        nc.sync.dma_start(zq[h1:h1 + 1, :], ctu_o[64:65, :])

                    # build deferred normalize + project for this qb
                    def mk_recip(zq=zq):
                        def run():
                            rc = z_pool.tile([NH, QB], FR, tag="rc", name="rc")
                            with nc.allow_low_precision(reason="softmax recip"):
                                nc.vector.reciprocal(rc[:], zq[:])
                            mk_recip.rc = rc
                        return run
                    rec = mk_recip()
                    pending.append(rec)

                    def mk_norm(pp, ctu=ctu_qb, rec=rec):
                        def run():
                            rc = mk_recip.rc
                            for h in (2 * pp, 2 * pp + 1):
                                bc = bc_ps.tile([64, QB], F32, tag="bc", name="bc")
                                nc.tensor.matmul(
                                    bc[:],
                                    lhsT=onehot[:, h * 64:(h + 1) * 64],
                                    rhs=rc[:], start=True, stop=True)
                                nc.vector.tensor_tensor(
                                    ctu[h][0:64, :], ctu[h][0:64, :], bc[:],
                                    mybir.AluOpType.mult)
                        return run
                    for pp in range(NPAIR):
                        pending.append(mk_norm(pp))

                    def mk_proj(mt, half, ctu=ctu_qb):
                        def run():
                            po = po_ps.tile([P, 512], F32, tag="po", name="po")
                            ms = slice((mt % 4) * P, (mt % 4 + 1) * P)
                            for h in range(NH):
                                nc.tensor.matmul(
                                    po[:],
                                    lhsT=ctu[h][0:64, ms],
                                    rhs=wo_h[h][:, half * 512:(half + 1) * 512],
                                    start=(h == 0), stop=(h == NH - 1))
                            po_sb = po_pool.tile([P, 512], F32, tag="posb",
                                                 name="po_sb")
                            nc.vector.tensor_copy(po_sb[:], po[:])
                            nc.sync.dma_start(
                                out[mt * P:(mt + 1) * P,
                                    half * 512:(half + 1) * 512],
                                po_sb[:])
                        return run
                    for mtl in range(4):
                        for half in range(2):
                            pending.append(mk_proj(qb * 4 + mtl, half))

                # flush remaining deferred work (last qb)
                while pending:
                    emit_pending()
    nc.compile()
    return nc


_NC_CACHE = {}


def _get_nc(S=2048):
    if S not in _NC_CACHE:
        _NC_CACHE[S] = build_nc(S)
    return _NC_CACHE[S]


def kernel(x, Wq, Wk, Wv, Wo, bo):
    from concourse.bass_utils import run_bass_kernel_spmd

    x = np.asarray(x, dtype=np.float32)
    Wq = np.asarray(Wq, dtype=np.float32)
    Wk = np.asarray(Wk, dtype=np.float32)
    Wv = np.asarray(Wv, dtype=np.float32)
    Wo = np.asarray(Wo, dtype=np.float32)
    bo = np.asarray(bo, dtype=np.float32)

    bs, S, d = x.shape
    nc = _get_nc(S)

    sel = np.kron(np.eye(NH, dtype=np.float32), np.ones((1, 64), np.float32))
    in_maps = []
    for c in range(8):
        b, g = divmod(c, 2)
        cols = slice(g * HD, (g + 1) * HD)
        in_maps.append({
            "xT": np.ascontiguousarray(x[b].T),
            "wq": np.ascontiguousarray(Wq[:, cols]),
            "wk": np.ascontiguousarray(Wk[:, cols]),
            "wv": np.ascontiguousarray(Wv[:, cols]),
            "wo": np.ascontiguousarray(Wo[cols, :]),
            "sel": sel,
        })

    res = run_bass_kernel_spmd(nc, in_maps, core_ids=list(range(8)))
    outp = np.empty((bs, S, d), dtype=np.float32)
    for b in range(bs):
        outp[b] = res.results[2 * b]["out"] + res.results[2 * b + 1]["out"] + bo
    return outp


# revision 6
# speedup vs baseline: 1.3822x; 1.1209x over previous
"""Multi-head self-attention Trainium2 kernel (8 NeuronCores).

Sharding: 8 cores = 4 batches x 2 head-groups (8 heads each).
Core c handles batch b=c//2, heads [g*8, (g+1)*8) where g=c%2.
Each core computes a partial output (its heads' contribution to the
output projection); the host sums the two partials per batch and adds bo.

All matmul operands are bf16 (PSUM accumulation stays fp32, the exp
reads fp32 scores from PSUM). bf16 streams at the same 1 cycle/row as
fp32r but at much lower PE power: sustained fp32r puts the chip into a
~55% K=4/8 firmware duty-cycle after ~90us, while bf16 runs cooler.
It also halves DMA/SBUF traffic and doubles DVE copy throughput.

Per-core dataflow:
  xT [1024, 2048] (= x[b].T), wq/wk/wv [1024, 512], wo [512, 1024]
  A1: QT[p]/KT[p] = w_p.T @ x.T  [128, 2048] per head-pair p (2 heads x 64
      dims on partitions). PSUM accum over 8 k-tiles.
  A2: VS[jt] = [x_jt @ wv | ones] per 128-token tile: [128, 8*65] with a
      ones column per head (the ones column makes the PV matmul emit the
      softmax normalizer as row 64 of the context tile).
  B:  per (pair p, 512-query block qb), per key tile jt:
        ST pair: two concurrent row-group matmuls (K=64 at array rows 0-63
        and 64-127) -> st [128, 2, 512] PSUM (2 banks)
        PT = exp(0.125 * ST)  (one ScalarE act over 1024 cols, bf16 out)
        PV: ct_par[65, 512] += VS[jt]_h.T @ PT slot  (row 64 = sum of exp)
      pair tail: copy ct -> SBUF (ctu, 65 rows); DMA row 64 into a packed
      Z tile [8, 512] (one row per head).
  Normalize+project for block qb are DEFERRED and emitted interleaved
  into block qb+1's attention stream (keeps the PE dense):
      one batched reciprocal [8, 512] per qb, one-hot K=8 matmul
      broadcast -> bc [64,512], DVE multiply (in place on ctu),
      then out[tokens, :] accumulated over 8 heads (K=64) per mt/half.
"""

import numpy as np
import ml_dtypes

import concourse.bass as bass
import concourse.tile as tile
from concourse import bacc, mybir
from contextlib import ExitStack

P = 128
D = 1024
HD = 512  # head dims per core (8 heads x 64)
NPAIR = 4
NH = 8
F32 = mybir.dt.float32
BF = mybir.dt.bfloat16
BF_NP = ml_dtypes.bfloat16


def build_nc(S=2048):
    NKT = D // P          # 8 k-tiles over model dim
    NJT = S // P          # 16 key tiles
    MSEG = 512
    NMSEG = S // MSEG
    QB = 512
    NQB = S // QB

    nc = bacc.Bacc("TRN2", target_bir_lowering=False, debug=False)
    xT = nc.dram_tensor("xT", [D, S], BF, kind="ExternalInput").ap()
    wq = nc.dram_tensor("wq", [D, HD], BF, kind="ExternalInput").ap()
    wk = nc.dram_tensor("wk", [D, HD], BF, kind="ExternalInput").ap()
    wv = nc.dram_tensor("wv", [D, HD], BF, kind="ExternalInput").ap()
    wo = nc.dram_tensor("wo", [HD, D], BF, kind="ExternalInput").ap()
    sel = nc.dram_tensor("sel", [NH, NH * 64], BF, kind="ExternalInput").ap()
    out = nc.dram_tensor("out", [S, D], F32, kind="ExternalOutput").ap()

    with tile.TileContext(nc) as tc:
        with ExitStack() as persist:
            const_pool = persist.enter_context(tc.tile_pool(name="const", bufs=1))
            data_pool = persist.enter_context(tc.tile_pool(name="data", bufs=1))

            # one-hot selector rows for the Z-broadcast matmul:
            # onehot[j, h*64+m] = (j == h), so lhsT=onehot[:, h*64:(h+1)*64]
            # with rhs=rc[8, 512] broadcasts rc row h onto 64 partitions.
            # (loaded from host: engines can't write at partition offsets 1-7)
            onehot = const_pool.tile([NH, NH * 64], BF, tag="oh", name="onehot")
            nc.sync.dma_start(onehot[:], sel[:])
            ones8_f32 = const_pool.tile([P, NH], F32, tag="ones8", name="ones8_f32")
            nc.vector.memset(ones8_f32[:], 1.0)

            QT = [data_pool.tile([P, S], BF, tag=f"qt{p}", name=f"qt{p}")
                  for p in range(NPAIR)]
            KT = [data_pool.tile([P, S], BF, tag=f"kt{p}", name=f"kt{p}")
                  for p in range(NPAIR)]
            # [128 tokens, 8 heads x (64 dims + ones col)]
            VS = [data_pool.tile([P, NH * 65], BF, tag=f"vs{j}", name=f"vs{j}")
                  for j in range(NJT)]

            # ---------------- Phase A: projections ----------------
            with ExitStack() as es_a:
                w_pool = es_a.enter_context(tc.tile_pool(name="wpool", bufs=1))
                chunk_pool = es_a.enter_context(tc.tile_pool(name="chunks", bufs=6))

                # per-kt weight tiles so the first matmuls start after ~3
                # small DMAs instead of 3 x 2MB ones
                wq_t, wk_t, wv_t = [], [], []
                for kt in range(NKT):
                    tq = w_pool.tile([P, HD], BF, tag=f"wq{kt}", name=f"wq{kt}")
                    nc.sync.dma_start(tq[:], wq[kt * P:(kt + 1) * P, :])
                    wq_t.append(tq)
                    tk = w_pool.tile([P, HD], BF, tag=f"wk{kt}", name=f"wk{kt}")
                    nc.sync.dma_start(tk[:], wk[kt * P:(kt + 1) * P, :])
                    wk_t.append(tk)
                for kt in range(NKT):
                    tv = w_pool.tile([P, HD], BF, tag=f"wv{kt}", name=f"wv{kt}")
                    nc.sync.dma_start(tv[:], wv[kt * P:(kt + 1) * P, :])
                    wv_t.append(tv)

                # --- A1: QT / KT (8 PSUM accumulators: (q|k) x 4 pairs) ---
                with tc.tile_pool(name="qkps", bufs=8, space="PSUM") as qk_pool:
                    for mseg in range(NMSEG):
                        accs = [qk_pool.tile([P, MSEG], F32, tag="qk", name="qkacc")
                                for _ in range(8)]
                        for kt in range(NKT):
                            xc = chunk_pool.tile([P, MSEG], BF, tag="xc", name="xc")
                            nc.sync.dma_start(
                                xc[:],
                                xT[kt * P:(kt + 1) * P, mseg * MSEG:(mseg + 1) * MSEG])
                            for p in range(NPAIR):
                                for ti, wt in ((0, wq_t), (1, wk_t)):
                                    nc.tensor.matmul(
                                        accs[p * 2 + ti][:],
                                        lhsT=wt[kt][:, p * P:(p + 1) * P],
                                        rhs=xc[:],
                                        start=(kt == 0), stop=(kt == NKT - 1))
                        for p in range(NPAIR):
                            nc.vector.tensor_copy(
                                QT[p][:, mseg * MSEG:(mseg + 1) * MSEG], accs[p * 2][:])
                            nc.vector.tensor_copy(
                                KT[p][:, mseg * MSEG:(mseg + 1) * MSEG], accs[p * 2 + 1][:])

                # --- A2: V (natural layout, 4 j-tiles per mseg) ---
                with tc.tile_pool(name="vps", bufs=8, space="PSUM") as v_pool:
                    for mseg in range(NMSEG):
                        vaccs = [v_pool.tile([P, HD], F32, tag="v", name="vacc")
                                 for _ in range(4)]
                        for kt in range(NKT):
                            xc = chunk_pool.tile([P, MSEG], BF, tag="xc", name="xc")
                            nc.sync.dma_start(
                                xc[:],
                                xT[kt * P:(kt + 1) * P, mseg * MSEG:(mseg + 1) * MSEG])
                            for i in range(4):
                                nc.tensor.matmul(
                                    vaccs[i][:],
                                    lhsT=xc[:, i * P:(i + 1) * P],
                                    rhs=wv_t[kt][:],
                                    start=(kt == 0), stop=(kt == NKT - 1))
                        for i in range(4):
                            vsv = VS[mseg * 4 + i].rearrange("p (h c) -> p h c", c=65)
                            nc.vector.tensor_copy(vsv[:, :, 0:64], vaccs[i][:])
                            nc.vector.tensor_copy(vsv[:, :, 64], ones8_f32[:])

            # ---------------- Phases B + C: attention + projection ----------------
            with ExitStack() as es_b:
                wo_pool = es_b.enter_context(tc.tile_pool(name="wopool", bufs=1))
                pt_pool = es_b.enter_context(tc.tile_pool(name="ptpool", bufs=3))
                ctu_pool = es_b.enter_context(tc.tile_pool(name="ctupool", bufs=2))
                z_pool = es_b.enter_context(tc.tile_pool(name="zpool", bufs=2))
                po_pool = es_b.enter_context(tc.tile_pool(name="popool", bufs=3))
                st_ps = es_b.enter_context(tc.tile_pool(name="stps", bufs=2, space="PSUM"))
                ct_ps = es_b.enter_context(tc.tile_pool(name="ctps", bufs=1, space="PSUM"))
                bc_ps = es_b.enter_context(tc.tile_pool(name="bcps", bufs=1, space="PSUM"))
                po_ps = es_b.enter_context(tc.tile_pool(name="pops", bufs=1, space="PSUM"))

                wo_h = []
                for h in range(NH):
                    t = wo_pool.tile([64, D], BF, tag=f"wo{h}", name=f"wo{h}")
                    nc.sync.dma_start(t[:], wo[h * 64:(h + 1) * 64, :])
                    wo_h.append(t)

                pending = []  # deferred normalize+project chunks (closures)

                def emit_pending():
                    if pending:
                        pending.pop(0)()

                for qb in range(NQB):
                    zq = z_pool.tile([NH, QB], BF, tag="z", name="zq")
                    ctu_qb = [None] * NH
                    for p in range(NPAIR):
                        h0, h1 = 2 * p, 2 * p + 1
                        cte = ct_ps.tile([65, QB], F32, tag="cte", name="cte")
                        cto = ct_ps.tile([65, QB], F32, tag="cto", name="cto")
                        qs = slice(qb * QB, (qb + 1) * QB)
                        for jt in range(NJT):
                            js = slice(jt * P, (jt + 1) * P)
                            stg = st_ps.tile([P, 2, MSEG], F32, tag="st", name="stg")
                            # two concurrent row-group matmuls (K=64)
                            nc.tensor.matmul(
                                stg[:, 0, :],
                                lhsT=KT[p][0:64, js], rhs=QT[p][0:64, qs],
                                start=True, stop=True)
                            nc.tensor.matmul(
                                stg[:, 1, :],
                                lhsT=KT[p][64:128, js], rhs=QT[p][64:128, qs],
                                start=True, stop=True)
                            ptg = pt_pool.tile([P, 2, MSEG], BF, tag="pt", name="ptg")
                            nc.scalar.activation(
                                ptg[:], stg[:],
                                mybir.ActivationFunctionType.Exp, scale=0.125)
                            nc.tensor.matmul(
                                cte[:],
                                lhsT=VS[jt][:, h0 * 65:(h0 + 1) * 65],
                                rhs=ptg[:, 0, :],
                                start=(jt == 0), stop=(jt == NJT - 1))
                            nc.tensor.matmul(
                                cto[:],
                                lhsT=VS[jt][:, h1 * 65:(h1 + 1) * 65],
                                rhs=ptg[:, 1, :],
                                start=(jt == 0), stop=(jt == NJT - 1))
                            if jt % 2 == 1 and p < 2:
                                emit_pending()
                        # pair tail: pull context (and Z rows) out of PSUM
                        ctu_e = ctu_pool.tile([65, QB], BF, tag=f"ctu{h0}",
                                              name=f"ctu{h0}")
                        nc.vector.tensor_copy(ctu_e[:], cte[:])
                        ctu_o = ctu_pool.tile([65, QB], BF, tag=f"ctu{h1}",
                                              name=f"ctu{h1}")
                        nc.vector.tensor_copy(ctu_o[:], cto[:])
                        ctu_qb[h0], ctu_qb[h1] = ctu_e, ctu_o
                        # pack Z rows (partition 64 -> partition h) for one
                        # batched reciprocal per qb
                        nc.sync.dma_start(zq[h0:h0 + 1, :], ctu_e[64:65, :])
                        nc.sync.dma_start(zq[h1:h1 + 1, :], ctu_o[64:65, :])

                    # build deferred normalize + project for this qb
                    def mk_recip(zq=zq):
                        def run():
                            rc = z_pool.tile([NH, QB], BF, tag="rc", name="rc")
                            with nc.allow_low_precision(reason="softmax recip"):
                                nc.vector.reciprocal(rc[:], zq[:])
                            mk_recip.rc = rc
                        return run
                    rec = mk_recip()
                    pending.append(rec)

                    def mk_norm(pp, ctu=ctu_qb):
                        def run():
                            rc = mk_recip.rc
                            for h in (2 * pp, 2 * pp + 1):
                                bc = bc_ps.tile([64, QB], F32, tag="bc", name="bc")
                                nc.tensor.matmul(
                                    bc[:],
                                    lhsT=onehot[:, h * 64:(h + 1) * 64],
                                    rhs=rc[:], start=True, stop=True)
                                nc.vector.tensor_tensor(
                                    ctu[h][0:64, :], ctu[h][0:64, :], bc[:],
                                    mybir.AluOpType.mult)
                        return run
                    for pp in range(NPAIR):
                        pending.append(mk_norm(pp))

                    def mk_proj(mt, half, ctu=ctu_qb):
                        def run():
                            po = po_ps.tile([P, 512], F32, tag="po", name="po")
                            ms = slice((mt % 4) * P, (mt % 4 + 1) * P)
                            for h in range(NH):
                                nc.tensor.matmul(
                                    po[:],
                                    lhsT=ctu[h][0:64, ms],
                                    rhs=wo_h[h][:, half * 512:(half + 1) * 512],
                                    start=(h == 0), stop=(h == NH - 1))
                            po_sb = po_pool.tile([P, 512], F32, tag="posb",
                                                 name="po_sb")
                            nc.vector.tensor_copy(po_sb[:], po[:])
                            nc.sync.dma_start(
                                out[mt * P:(mt + 1) * P,
                                    half * 512:(half + 1) * 512],
                                po_sb[:])
                        return run
                    for mtl in range(4):
                        for half in range(2):
                            pending.append(mk_proj(qb * 4 + mtl, half))

                # flush remaining deferred work (last qb)
                while pending:
                    emit_pending()
    nc.compile()
    return nc


_NC_CACHE = {}


def _get_nc(S=2048):
    if S not in _NC_CACHE:
        _NC_CACHE[S] = build_nc(S)
    return _NC_CACHE[S]


def _bf(a):
    return np.ascontiguousarray(a.astype(BF_NP))


def make_in_maps(x, Wq, Wk, Wv, Wo):
    sel = np.kron(np.eye(NH, dtype=np.float32), np.ones((1, 64), np.float32))
    sel = _bf(sel)
    in_maps = []
    for c in range(8):
        b, g = divmod(c, 2)
        cols = slice(g * HD, (g + 1) * HD)
        in_maps.append({
            "xT": _bf(x[b].T),
            "wq": _bf(Wq[:, cols]),
            "wk": _bf(Wk[:, cols]),
            "wv": _bf(Wv[:, cols]),
            "wo": _bf(Wo[cols, :]),
            "sel": sel,
        })
    return in_maps


def kernel(x, Wq, Wk, Wv, Wo, bo):
    from concourse.bass_utils import run_bass_kernel_spmd

    x = np.asarray(x, dtype=np.float32)
    Wq = np.asarray(Wq, dtype=np.float32)
    Wk = np.asarray(Wk, dtype=np.float32)
    Wv = np.asarray(Wv, dtype=np.float32)
    Wo = np.asarray(Wo, dtype=np.float32)
    bo = np.asarray(bo, dtype=np.float32)

    bs, S, d = x.shape
    nc = _get_nc(S)
    in_maps = make_in_maps(x, Wq, Wk, Wv, Wo)

    res = run_bass_kernel_spmd(nc, in_maps, core_ids=list(range(8)))
    outp = np.empty((bs, S, d), dtype=np.float32)
    for b in range(bs):
        outp[b] = res.results[2 * b]["out"] + res.results[2 * b + 1]["out"] + bo
    return outp
